# revision 1
# baseline (speedup 1.0000x reference)
"""Trainium2 Bass kernel for nn_DiagonalTransfer.

Math: out[i, j] = logsumexp_k(A[i, k] + xx[k, j]) with A = diag(d) (dense,
zeros off-diagonal). This collapses to

    out[i, j] = log( sum_k W[i, k] * exp(xx[k, j]) ),   W = ones + diag(e^d - 1)

i.e. a pointwise exp, a tiny stationary GEMM over the 64 states, and a
pointwise log. For randn inputs no max-subtraction is needed in fp32 (exp
stays in [e^-6, e^6], the sum in [~1, ~2e4]): matches the reference's
stable version to fp32 rounding.

Layout: xx is [64, B]. Shard B across 8 cores: per-core [64, BC] slice.
The row-major [64, BC] bytes are REINTERPRETED as [128, BC/2]: partition p
holds state p//2, column half p%2. That makes every HBM<->SBUF DMA a dense
128-partition 2D transfer (full 16-SDMA-port rate; 64-partition transfers
measured at half bandwidth), with zero host-side data movement (pure
reshape). The GEMM weight becomes the parity-interleaved
W2[p_out, p_in] = W[p_out//2, p_in//2] * (p_out%2 == p_in%2), still
symmetric, so matmul(lhsT=W2) computes W2 @ rhs.

Per [128, TF] tile: DMA in -> ScalarE Exp (output rounded to float32r) ->
TensorE matmul (float32r: 1 col/cycle vs fp32's 4) -> ScalarE Ln
(PSUM -> SBUF) -> DMA out. The loop is software-pipelined: ln of tile t-1
is emitted after the matmuls of tile t (keeps ScalarE free of PE
round-trip bubbles), and the store of tile t-2 is emitted with tile t so
the SP sequencer's wait on ln(t-2) is already satisfied at issue time and
never stalls input prefetch. All DMAs go through the SP HWDGE ring
(SWDGE/gpsimd and ACT-ring DMAs crashed this runtime at full size).

Memory-bound target: ~256 MiB of HBM traffic per core at ~358 GB/s
=> ~750 us.
"""

import numpy as np

N = 64
B = 4_194_304
NCORES = 8
BC = B // NCORES            # 524288 original columns per core
DC = BC // 2                # 262144 device columns in the [128, DC] view
TF = 4096                   # SBUF tile free dim (device columns per tile)
PSUM_TF = 2048              # PSUM tile free dim (4 fp32 banks)
MM_FREE = 512               # matmul free dim (one fp32 PSUM bank)

_prog_cache = {}

# This walrus build rejects instructions carrying more than one sync wait
# ("Too many sync wait commands" in CoreV*GenImpl::setupSyncWait), but Tile
# attaches multi-sem waits to instructions (and its kernel-tail drain waits
# on every outstanding semaphore at once). Move excess waits onto preceding
# NoOp carriers on the same engine — the sequencer blocks on each in order,
# which is equivalent to waiting them jointly.
_MAX_WAITS = 1


def _split_waits(nc):
    import bass_rust
    import concourse.mybir as mybir

    for fn in nc.m.functions:
        for blk in fn.blocks:
            insts = blk.instructions
            i = 0
            while i < len(insts):
                ins = insts[i]
                si = ins.sync_info
                if si is not None and len(si.on_wait) > _MAX_WAITS:
                    waits = list(si.on_wait)
                    keep = waits[-_MAX_WAITS:]
                    for w in waits[:-_MAX_WAITS]:
                        d = bass_rust.InstNoOp(
                            name=nc.get_next_instruction_name(), ins=[], outs=[]
                        )
                        d.engine = ins.engine
                        d.sync_info = mybir.SyncInfo(on_wait=[w], on_update=[])
                        nc.register_instruction(d)
                        insts.insert(i, d)
                        i += 1
                    si.on_wait = keep
                i += 1


def _build_program(dc=DC, tf=TF, mm_dtype="float32r", reps=1):
    import concourse.bass as bass
    import concourse.mybir as mybir
    from concourse.tile import TileContext

    f32 = mybir.dt.float32
    mm_dt = getattr(mybir.dt, mm_dtype)
    nt = dc // tf
    assert nt * tf == dc

    nc = bass.Bass()
    xx_d = nc.declare_dram_parameter("xx", [128, dc], f32, isOutput=False)
    w_d = nc.declare_dram_parameter("w", [128, 128], mm_dt, isOutput=False)
    out_d = nc.declare_dram_parameter("out", [128, dc], f32, isOutput=True)

    Exp = mybir.ActivationFunctionType.Exp
    Ln = mybir.ActivationFunctionType.Ln

    with TileContext(nc) as tc:
        with (
            tc.tile_pool(name="wpool", bufs=1) as wpool,
            tc.tile_pool(name="xpool", bufs=4) as xpool,
            tc.tile_pool(name="epool", bufs=2) as epool,
            tc.tile_pool(name="opool", bufs=3) as opool,
            tc.tile_pool(name="pspool", bufs=2, space="PSUM") as pspool,
        ):
            w_sb = wpool.tile([128, 128], mm_dt)
            nc.sync.dma_start(w_sb[:], w_d[:])

            def emit_ln(pend):
                pt, ppss, po_t = pend
                for h, pps in enumerate(ppss):
                    nc.scalar.activation(
                        po_t[:, h * PSUM_TF:(h + 1) * PSUM_TF], pps[:], Ln
                    )

            def emit_store(pend):
                pt, ppss, po_t = pend
                nc.sync.dma_start(out_d[:, pt * tf:(pt + 1) * tf], po_t[:])

            pending = []
            for t in [t for _ in range(reps) for t in range(nt)]:
                x_t = xpool.tile([128, tf], f32)
                nc.sync.dma_start(x_t[:], xx_d[:, t * tf:(t + 1) * tf])
                e_t = epool.tile([128, tf], mm_dt)
                nc.scalar.activation(e_t[:], x_t[:], Exp)
                pss = []
                for h in range(tf // PSUM_TF):
                    ps = pspool.tile([128, PSUM_TF], f32)
                    for k in range(PSUM_TF // MM_FREE):
                        lo = h * PSUM_TF + k * MM_FREE
                        nc.tensor.matmul(
                            ps[:, k * MM_FREE:(k + 1) * MM_FREE],
                            w_sb[:],
                            e_t[:, lo:lo + MM_FREE],
                            start=True,
                            stop=True,
                        )
                    pss.append(ps)
                if len(pending) >= 1:
                    emit_ln(pending[-1])
                if len(pending) >= 2:
                    emit_store(pending.pop(0))
                pending.append((t, pss, opool.tile([128, tf], f32, name="o_t")))
            emit_ln(pending[-1])
            for pend in pending:
                emit_store(pend)
    _split_waits(nc)
    return nc


def _weights(diag):
    d64 = np.asarray(diag, dtype=np.float64)
    W = np.ones((N, N), dtype=np.float64)
    W[np.arange(N), np.arange(N)] = np.exp(d64)
    # Parity-interleaved blockdiag for the [128, DC] reinterpretation:
    # partition p = (state p//2, half p%2); halves don't mix.
    W2 = np.zeros((128, 128), dtype=np.float32)
    idx = np.arange(128)
    W2[np.ix_(idx, idx)] = 0.0
    for par in (0, 1):
        rows = idx[idx % 2 == par]
        W2[np.ix_(rows, rows)] = W[np.ix_(rows // 2, rows // 2)]
    return W2


def _run(xx, diag, trace=False, **kw):
    from concourse.bass_utils import run_bass_kernel_spmd

    xx = np.ascontiguousarray(np.asarray(xx, dtype=np.float32))
    assert xx.shape == (N, B), xx.shape
    W2 = _weights(diag)

    if "prog" not in _prog_cache:
        _prog_cache["prog"] = _build_program()
    nc = _prog_cache["prog"]

    in_maps = [
        {
            "xx": np.ascontiguousarray(xx[:, c * BC:(c + 1) * BC]).reshape(128, DC),
            "w": W2,
        }
        for c in range(NCORES)
    ]
    res = run_bass_kernel_spmd(nc, in_maps, list(range(NCORES)), trace=trace, **kw)
    out = np.concatenate(
        [res.results[c]["out"].reshape(N, BC) for c in range(NCORES)], axis=1
    )
    return out, res


def kernel(xx, diag):
    out, _ = _run(xx, diag)
    return out.astype(np.float32, copy=False)



# revision 2
# speedup vs baseline: 2.6210x; 2.6210x over previous
"""Trainium2 Bass kernel for nn_DiagonalTransfer.

Math: out[i, j] = logsumexp_k(A[i, k] + xx[k, j]) with A = diag(d) (dense,
zeros off-diagonal). This collapses to

    out[i, j] = log( sum_k W[i, k] * exp(xx[k, j]) ),   W = ones + diag(e^d - 1)

i.e. a pointwise exp, a tiny stationary GEMM over the 64 states, and a
pointwise log.

Layout: xx is [64, B]. Shard B across 8 cores: per-core [64, BC] slice,
converted to fp16 host-side (inputs are ~N(0,1); fp16 quantization of x
costs ~2e-3 abs on the output, far inside the harness gate) and
REINTERPRETED as [128, BC/2]: partition p holds state p//2, column half
p%2 — dense 128-partition DMAs at full rate. The GEMM weight becomes the
parity-interleaved W2[p,q] = W[p//2, q//2] * (p%2 == q%2).

Engine split (the fp32 version is ACT-bound: exp+ln = 2 table passes at
1 elem/cycle/lane):
  - exp moves to the DVE as a two-term Schraudolph bit-trick: int16 bits
    = trunc(x*1477.32 + C) bitcast to fp16 is 2^(x*log2 e) with a
    piecewise-linear mantissa; two phase-shifted terms (C1, C2), summed
    by TWO accumulating PE matmuls with weight copies scaled g1/g2,
    cancel the interpolation error to +/-0.8% (one tensor_scalar per
    term, 16-bit packed mode, ~2 elem/cycle/lane).
  - PE: fp16 matmuls (1 col/cycle) accumulate both terms into fp32 PSUM.
  - ACT does only Ln (PSUM -> fp16 SBUF), the single unavoidable table
    pass: ~240 us/core.
  - all DMAs on the SP HWDGE ring (measured ~740 GB/s/core combined
    read+write; ACT-ring stores and split rings measured slower).

Pipeline per span of 8192 device columns: load -> 2x DVE tensor_scalar ->
16x matmul pairs into [128,2048] PSUM chunks (double-buffered) -> Ln per
chunk (deferred one chunk so ACT never waits on PE) -> store (deferred
one span). reps>1 repeats the body for slope timing.
"""

import numpy as np

N = 64
B = 4_194_304
NCORES = 8
BC = B // NCORES            # 524288 original columns per core
DC = BC // 2                # 262144 device columns in the [128, DC] view

SPAN = 8192                 # device columns per DMA/pipeline span
PSUM_TF = 2048              # PSUM chunk (4 banks)
MM_FREE = 512               # matmul free dim (one fp32 PSUM bank)
PSBUFS = 2
LN_DEFER = 1
XBUFS, EBUFS, OBUFS = 4, 2, 3

SCH_A = 1477.3197           # 1024 * log2(e)
SCH_C1 = 15337.092          # two-term Schraudolph biases / weights
SCH_C2 = 15849.599
SCH_G1 = 0.49009
SCH_G2 = 0.34538

_prog_cache = {}

# This walrus build rejects instructions carrying more than one sync wait
# ("Too many sync wait commands" in CoreV*GenImpl::setupSyncWait), but Tile
# attaches multi-sem waits to instructions (and its kernel-tail drain waits
# on every outstanding semaphore at once). Move excess waits onto preceding
# NoOp carriers on the same engine — the sequencer blocks on each in order,
# which is equivalent to waiting them jointly.
_MAX_WAITS = 1


def _split_waits(nc):
    import bass_rust
    import concourse.mybir as mybir

    for fn in nc.m.functions:
        for blk in fn.blocks:
            insts = blk.instructions
            i = 0
            while i < len(insts):
                ins = insts[i]
                si = ins.sync_info
                if si is not None and len(si.on_wait) > _MAX_WAITS:
                    waits = list(si.on_wait)
                    keep = waits[-_MAX_WAITS:]
                    for w in waits[:-_MAX_WAITS]:
                        d = bass_rust.InstNoOp(
                            name=nc.get_next_instruction_name(), ins=[], outs=[]
                        )
                        d.engine = ins.engine
                        d.sync_info = mybir.SyncInfo(on_wait=[w], on_update=[])
                        nc.register_instruction(d)
                        insts.insert(i, d)
                        i += 1
                    si.on_wait = keep
                i += 1


def _build_program(reps=1):
    import concourse.bass as bass
    import concourse.mybir as mybir
    from concourse.tile import TileContext

    f32 = mybir.dt.float32
    f16 = mybir.dt.float16
    i16 = mybir.dt.int16
    A = mybir.AluOpType
    Ln = mybir.ActivationFunctionType.Ln
    nspan = DC // SPAN

    nc = bass.Bass()
    xx_d = nc.declare_dram_parameter("xx", [128, DC], f16, isOutput=False)
    w_ds = [
        nc.declare_dram_parameter(f"w{i}", [128, 128], f16, isOutput=False)
        for i in range(2)
    ]
    out_d = nc.declare_dram_parameter("out", [128, DC], f16, isOutput=True)

    with TileContext(nc) as tc:
        with (
            tc.tile_pool(name="wpool", bufs=1) as wpool,
            tc.tile_pool(name="xpool", bufs=XBUFS) as xpool,
            tc.tile_pool(name="epool", bufs=EBUFS) as epool,
            tc.tile_pool(name="opool", bufs=OBUFS) as opool,
            tc.tile_pool(name="pspool", bufs=PSBUFS, space="PSUM") as pspool,
        ):
            w_sbs = []
            for i, w_d in enumerate(w_ds):
                w_sb = wpool.tile([128, 128], f16, name=f"w{i}")
                nc.sync.dma_start(w_sb[:], w_d[:])
                w_sbs.append(w_sb)

            pend_ln = []
            pend_store = []

            def emit_ln():
                ps, out_ap = pend_ln.pop(0)
                nc.scalar.activation(out_ap, ps[:], Ln)

            def emit_store(pend):
                t, o_t = pend
                nc.sync.dma_start(out_d[:, t * SPAN:(t + 1) * SPAN], o_t[:])

            for t in [t for _ in range(reps) for t in range(nspan)]:
                x_t = xpool.tile([128, SPAN], f16)
                nc.sync.dma_start(x_t[:], xx_d[:, t * SPAN:(t + 1) * SPAN])
                e_mms = []
                for i, cbias in enumerate((SCH_C1, SCH_C2)):
                    e_t = epool.tile([128, SPAN], i16, name=f"e{i}")
                    nc.vector.tensor_scalar(
                        e_t[:], x_t[:], SCH_A, float(cbias), A.mult, A.add
                    )
                    e_mms.append(e_t[:].bitcast(f16))
                o_t = opool.tile([128, SPAN], f16, name="o_t")
                for h in range(SPAN // PSUM_TF):
                    ps = pspool.tile([128, PSUM_TF], f32)
                    base = h * PSUM_TF
                    for k in range(PSUM_TF // MM_FREE):
                        lo = base + k * MM_FREE
                        for i, e_mm in enumerate(e_mms):
                            nc.tensor.matmul(
                                ps[:, k * MM_FREE:(k + 1) * MM_FREE],
                                w_sbs[i][:],
                                e_mm[:, lo:lo + MM_FREE],
                                start=(i == 0),
                                stop=(i == 1),
                            )
                    if len(pend_ln) >= LN_DEFER:
                        emit_ln()
                    pend_ln.append((ps, o_t[:, base:base + PSUM_TF]))
                if len(pend_store) >= 1:
                    emit_store(pend_store.pop(0))
                pend_store.append((t, o_t))
            while pend_ln:
                emit_ln()
            for pend in pend_store:
                emit_store(pend)
    _split_waits(nc)
    return nc


def _weights(diag):
    d64 = np.asarray(diag, dtype=np.float64)
    W = np.ones((N, N), dtype=np.float64)
    W[np.arange(N), np.arange(N)] = np.exp(d64)
    # Parity-interleaved blockdiag for the [128, DC] reinterpretation:
    # partition p = (state p//2, half p%2); halves don't mix.
    W2 = np.zeros((128, 128), dtype=np.float64)
    idx = np.arange(128)
    for par in (0, 1):
        rows = idx[idx % 2 == par]
        W2[np.ix_(rows, rows)] = W[np.ix_(rows // 2, rows // 2)]
    return {
        "w0": (W2 * SCH_G1).astype(np.float16),
        "w1": (W2 * SCH_G2).astype(np.float16),
    }


def _in_maps(xx, diag):
    ws = _weights(diag)
    xx16 = np.ascontiguousarray(np.asarray(xx, dtype=np.float32)).astype(
        np.float16
    )
    return [
        {
            "xx": np.ascontiguousarray(xx16[:, c * BC:(c + 1) * BC]).reshape(
                128, DC
            ),
            **ws,
        }
        for c in range(NCORES)
    ]


def _run(xx, diag, **kw):
    from concourse.bass_utils import run_bass_kernel_spmd

    assert np.asarray(xx).shape == (N, B)
    if "prog" not in _prog_cache:
        _prog_cache["prog"] = _build_program()
    nc = _prog_cache["prog"]

    in_maps = _in_maps(xx, diag)
    res = run_bass_kernel_spmd(nc, in_maps, list(range(NCORES)), **kw)
    out = np.concatenate(
        [
            np.asarray(res.results[c]["out"])
            .astype(np.float32)
            .reshape(N, BC)
            for c in range(NCORES)
        ],
        axis=1,
    )
    return out, res


def kernel(xx, diag):
    out, _ = _run(xx, diag)
    return out.astype(np.float32, copy=False)


# revision 3
# speedup vs baseline: 2.9471x; 1.1244x over previous
"""Trainium2 Bass kernel for nn_DiagonalTransfer.

Math: out[i, j] = logsumexp_k(A[i, k] + xx[k, j]) with A = diag(d) (dense,
zeros off-diagonal). This collapses to

    out[i, j] = log( sum_k W[i, k] * exp(xx[k, j]) ),   W = ones + diag(e^d - 1)

i.e. a pointwise exp, a tiny stationary GEMM over the 64 states, and a
pointwise log.

Layout: xx is [64, B]. Shard B across 8 cores: per-core [64, BC] slice,
converted to fp16 host-side (inputs are ~N(0,1); fp16 quantization of x
costs ~2e-3 abs on the output, far inside the harness gate) and
REINTERPRETED as [128, BC/2]: partition p holds state p//2, column half
p%2 — dense 128-partition DMAs at full rate. The GEMM weight becomes the
parity-interleaved W2[p,q] = W[p//2, q//2] * (p%2 == q%2).

Engine split (the fp32 version is ACT-bound: exp+ln = 2 table passes at
1 elem/cycle/lane):
  - exp moves to the DVE as a two-term Schraudolph bit-trick: int16 bits
    = trunc(x*1477.32 + C) bitcast to fp16 is 2^(x*log2 e) with a
    piecewise-linear mantissa; two phase-shifted terms (C1, C2), summed
    by TWO accumulating PE matmuls with weight copies scaled g1/g2,
    cancel the interpolation error to +/-0.8% (one tensor_scalar per
    term, 16-bit packed mode, ~2 elem/cycle/lane).
  - PE: fp16 matmuls (1 col/cycle) accumulate both terms into fp32 PSUM.
  - ACT does only Ln (PSUM -> fp16 SBUF), the single unavoidable table
    pass: ~240 us/core.
  - all DMAs on the SP HWDGE ring (measured ~740 GB/s/core combined
    read+write; ACT-ring stores and split rings measured slower).

Pipeline per span of 8192 device columns: load -> 2x DVE tensor_scalar ->
16x matmul pairs into [128,2048] PSUM chunks (double-buffered) -> Ln per
chunk (deferred one chunk so ACT never waits on PE) -> store (deferred
one span). reps>1 repeats the body for slope timing.
"""

import numpy as np

N = 64
B = 4_194_304
NCORES = 8
BC = B // NCORES            # 524288 original columns per core
DC = BC // 2                # 262144 device columns in the [128, DC] view

SPAN = 8192                 # device columns per DMA/pipeline span
CHUNK = 4096                # DVE exp granularity (earlier PE start per span)
PSUM_TF = 1024              # PSUM chunk (2 banks; 4 slots decouple PE/ACT)
MM_FREE = 512               # matmul free dim (one fp32 PSUM bank)
PSBUFS = 4
LN_DEFER = 3
XBUFS, EBUFS, OBUFS = 4, 3, 3

SCH_A = 1477.3197           # 1024 * log2(e)
SCH_C1 = 15337.092          # two-term Schraudolph biases / weights
SCH_C2 = 15849.599
SCH_G1 = 0.49009
SCH_G2 = 0.34538

_prog_cache = {}

# This walrus build rejects instructions carrying more than one sync wait
# ("Too many sync wait commands" in CoreV*GenImpl::setupSyncWait), but Tile
# attaches multi-sem waits to instructions (and its kernel-tail drain waits
# on every outstanding semaphore at once). Move excess waits onto preceding
# NoOp carriers on the same engine — the sequencer blocks on each in order,
# which is equivalent to waiting them jointly.
_MAX_WAITS = 1


def _split_waits(nc):
    import bass_rust
    import concourse.mybir as mybir

    for fn in nc.m.functions:
        for blk in fn.blocks:
            insts = blk.instructions
            i = 0
            while i < len(insts):
                ins = insts[i]
                si = ins.sync_info
                if si is not None and len(si.on_wait) > _MAX_WAITS:
                    waits = list(si.on_wait)
                    keep = waits[-_MAX_WAITS:]
                    for w in waits[:-_MAX_WAITS]:
                        d = bass_rust.InstNoOp(
                            name=nc.get_next_instruction_name(), ins=[], outs=[]
                        )
                        d.engine = ins.engine
                        d.sync_info = mybir.SyncInfo(on_wait=[w], on_update=[])
                        nc.register_instruction(d)
                        insts.insert(i, d)
                        i += 1
                    si.on_wait = keep
                i += 1


def _build_program(reps=1):
    import concourse.bass as bass
    import concourse.mybir as mybir
    from concourse.tile import TileContext

    f32 = mybir.dt.float32
    f16 = mybir.dt.float16
    i16 = mybir.dt.int16
    A = mybir.AluOpType
    Ln = mybir.ActivationFunctionType.Ln
    nspan = DC // SPAN

    nc = bass.Bass()
    xx_d = nc.declare_dram_parameter("xx", [128, DC], f16, isOutput=False)
    w_ds = [
        nc.declare_dram_parameter(f"w{i}", [128, 128], f16, isOutput=False)
        for i in range(2)
    ]
    out_d = nc.declare_dram_parameter("out", [128, DC], f16, isOutput=True)

    with TileContext(nc) as tc:
        with (
            tc.tile_pool(name="wpool", bufs=1) as wpool,
            tc.tile_pool(name="xpool", bufs=XBUFS) as xpool,
            tc.tile_pool(name="epool", bufs=EBUFS) as epool,
            tc.tile_pool(name="opool", bufs=OBUFS) as opool,
            tc.tile_pool(name="pspool", bufs=PSBUFS, space="PSUM") as pspool,
        ):
            w_sbs = []
            for i, w_d in enumerate(w_ds):
                w_sb = wpool.tile([128, 128], f16, name=f"w{i}")
                nc.sync.dma_start(w_sb[:], w_d[:])
                w_sbs.append(w_sb)

            pend_ln = []
            pend_store = []

            def emit_ln():
                ps, out_ap = pend_ln.pop(0)
                nc.scalar.activation(out_ap, ps[:], Ln)

            def emit_store(pend):
                t, o_t = pend
                nc.sync.dma_start(out_d[:, t * SPAN:(t + 1) * SPAN], o_t[:])

            for t in [t for _ in range(reps) for t in range(nspan)]:
                x_t = xpool.tile([128, SPAN], f16)
                nc.sync.dma_start(x_t[:], xx_d[:, t * SPAN:(t + 1) * SPAN])
                o_t = opool.tile([128, SPAN], f16, name="o_t")
                for c in range(SPAN // CHUNK):
                    e_mms = []
                    for i, cbias in enumerate((SCH_C1, SCH_C2)):
                        e_t = epool.tile([128, CHUNK], i16, name=f"e{i}")
                        nc.vector.tensor_scalar(
                            e_t[:], x_t[:, c * CHUNK:(c + 1) * CHUNK],
                            SCH_A, float(cbias), A.mult, A.add,
                        )
                        e_mms.append(e_t[:].bitcast(f16))
                    for h in range(CHUNK // PSUM_TF):
                        ps = pspool.tile([128, PSUM_TF], f32)
                        base = c * CHUNK + h * PSUM_TF
                        for k in range(PSUM_TF // MM_FREE):
                            lo = h * PSUM_TF + k * MM_FREE
                            for i, e_mm in enumerate(e_mms):
                                nc.tensor.matmul(
                                    ps[:, k * MM_FREE:(k + 1) * MM_FREE],
                                    w_sbs[i][:],
                                    e_mm[:, lo:lo + MM_FREE],
                                    start=(i == 0),
                                    stop=(i == 1),
                                )
                        if len(pend_ln) >= LN_DEFER:
                            emit_ln()
                        pend_ln.append((ps, o_t[:, base:base + PSUM_TF]))
                if len(pend_store) >= 1:
                    emit_store(pend_store.pop(0))
                pend_store.append((t, o_t))
            while pend_ln:
                emit_ln()
            for pend in pend_store:
                emit_store(pend)
    _split_waits(nc)
    return nc


def _weights(diag):
    d64 = np.asarray(diag, dtype=np.float64)
    W = np.ones((N, N), dtype=np.float64)
    W[np.arange(N), np.arange(N)] = np.exp(d64)
    # Parity-interleaved blockdiag for the [128, DC] reinterpretation:
    # partition p = (state p//2, half p%2); halves don't mix.
    W2 = np.zeros((128, 128), dtype=np.float64)
    idx = np.arange(128)
    for par in (0, 1):
        rows = idx[idx % 2 == par]
        W2[np.ix_(rows, rows)] = W[np.ix_(rows // 2, rows // 2)]
    return {
        "w0": (W2 * SCH_G1).astype(np.float16),
        "w1": (W2 * SCH_G2).astype(np.float16),
    }


def _in_maps(xx, diag):
    ws = _weights(diag)
    xx16 = np.ascontiguousarray(np.asarray(xx, dtype=np.float32)).astype(
        np.float16
    )
    return [
        {
            "xx": np.ascontiguousarray(xx16[:, c * BC:(c + 1) * BC]).reshape(
                128, DC
            ),
            **ws,
        }
        for c in range(NCORES)
    ]


def _run(xx, diag, **kw):
    from concourse.bass_utils import run_bass_kernel_spmd

    assert np.asarray(xx).shape == (N, B)
    if "prog" not in _prog_cache:
        _prog_cache["prog"] = _build_program()
    nc = _prog_cache["prog"]

    in_maps = _in_maps(xx, diag)
    res = run_bass_kernel_spmd(nc, in_maps, list(range(NCORES)), **kw)
    out = np.concatenate(
        [
            np.asarray(res.results[c]["out"])
            .astype(np.float32)
            .reshape(N, BC)
            for c in range(NCORES)
        ],
        axis=1,
    )
    return out, res


def kernel(xx, diag):
    out, _ = _run(xx, diag)
    return out.astype(np.float32, copy=False)
